# revision 10
# baseline (speedup 1.0000x reference)
"""Trainium2 Bass kernel for sliding-window unfold (im2col).

reference:  out = x[:, idx, :]  with idx[w, f] = w + f
  x:   [128, 4096, 4]  f32
  out: [128, 4065, 32, 4]  f32

out[b, w] (= 128 floats = 512 B) is the contiguous slice
x[b].flat[4w : 4w + 128]; HBM write bandwidth is the roofline.

Measured on TRN2 (trace analysis across runs):
  - a dma_start whose SBUF side spans EXACTLY 128 partitions is sprayed
    across all 16 SDMA engines (~26.6 GB/s each); DRAM->DRAM DMAs and
    other partition counts land entirely on ONE engine (E64).
  - every DMA queue (sync/scalar HWDGE + gpsimd SWDGE) dispatches
    descriptors at ~16 ns/descriptor; a 128-row DMA costs ~2.05 us of
    queue time regardless of row size.
  - expands are cheaper split: DVE 16-window half ~1.3 us + ACT
    15-window half ~1.9 us in parallel, vs 5.1/3.6 us monolithic.

Per batch b on each core (pure data parallel, 16 batches/core):
  1. load X[128, 248]: partition p holds x[b].flat[124p : 124p+248].
  2. expand X -> Y[128, 3968] (Y[p, 128j+i] = X[p, 4j+i]), split
     across DVE (16 windows) and ACT (15 windows) in parallel.
  3. store Y -> out[b] windows 0..3967 (15.5 KB runs per partition).
  4. tail windows 3937..4064 ride a 128-partition load+store pair
     (31 rows rewrite bulk output with identical bytes - fast path).

Two-stream stores (v4 evidence: splitting stores across two queues
equalizes all 16 engines - the single-queue "slow E79" straggler was a
queue artifact): even-b bulk stores go through gpsimd (SWDGE), odd-b
bulk stores through the scalar HWDGE queue as direct SBUF->DRAM DMAs.
All loads ride the sync queue, X loads compressed FIRST (X_b lands at
~2.05 us cadence, all expands and store gens queue up early - a dense
store FIFO avoids the pool/completion self-pacing that interleaved
loads caused), tail loads behind them.  Tail stores dispatch on the
scalar queue AFTER the odd bulk stores: by then their loads have
landed, and their packets drain inside the bulk stream instead of
dribbling at the end.
"""

import numpy as np

from concourse import bacc, mybir, tile
from concourse.bass_utils import run_bass_kernel_spmd

N_CORES = 8
B_FULL = 128
B = B_FULL // N_CORES  # 16 batches per core
S = 4096
C = 4
F = 32
W = S - F + 1    # 4065
FL = F * C       # 128 floats per window
XB = S * C       # 16384 floats per batch of x
OB = W * FL      # 520320 floats per batch of out
WPP = 31         # windows per partition in the bulk store
NBULK = 128 * WPP          # 3968 bulk windows per batch
NTAIL = W - NBULK          # 97 tail windows
YROW = WPP * FL            # 3968 floats per partition row
XROW = (WPP - 1) * C + FL  # 248 floats of x per partition
WSPLIT = 16                # windows expanded on DVE (rest on ACT)

_cache = {}


def build_nc():
    nc = bacc.Bacc("TRN2", target_bir_lowering=False)
    x = nc.dram_tensor("x", [B, S, C], mybir.dt.float32, kind="ExternalInput")
    out = nc.dram_tensor("out", [B, W, F, C], mybir.dt.float32, kind="ExternalOutput")

    with tile.TileContext(nc) as tc:
        with (
            tc.tile_pool(name="xp", bufs=10) as xp,
            tc.tile_pool(name="yp", bufs=11) as yp,
            tc.tile_pool(name="tp", bufs=16) as tp,
        ):
            # -- all loads on the sync queue: X loads compressed first
            #    (they gate everything), tail loads behind them.
            Xs = []
            for b in range(B):
                X = xp.tile([128, XROW], mybir.dt.float32)
                src = x[:].copy()
                src.ap = mybir.VecI64Pair([[WPP * C, 128], [1, XROW]])
                src.offset = b * XB
                nc.sync.dma_start(out=X[:, :], in_=src)
                Xs.append(X)

            TBs = []
            for b in range(B):
                TB = tp.tile([128, FL], mybir.dt.float32)
                srcT = x[:].copy()
                srcT.ap = mybir.VecI64Pair([[C, 128], [1, FL]])
                srcT.offset = b * XB + (NBULK - 31) * C
                nc.sync.dma_start(out=TB[:, :], in_=srcT)
                TBs.append(TB)

            # -- expand (split DVE || ACT) + two-stream bulk stores.
            for b in range(B):
                X = Xs[b]
                Y = yp.tile([128, YROW], mybir.dt.float32)

                srcA = X[:].copy()
                srcA.ap = mybir.VecI64Pair([[XROW, 128], [C, WSPLIT], [1, FL]])
                srcA.offset = 0
                dstA = Y[:].copy()
                dstA.ap = mybir.VecI64Pair([[YROW, 128], [FL, WSPLIT], [1, FL]])
                dstA.offset = 0
                nc.vector.tensor_copy(out=dstA, in_=srcA)

                srcB = X[:].copy()
                srcB.ap = mybir.VecI64Pair([[XROW, 128], [C, WPP - WSPLIT], [1, FL]])
                srcB.offset = WSPLIT * C
                dstB = Y[:].copy()
                dstB.ap = mybir.VecI64Pair([[YROW, 128], [FL, WPP - WSPLIT], [1, FL]])
                dstB.offset = WSPLIT * FL
                nc.scalar.copy(out=dstB, in_=srcB)

                dst3 = out[:].copy()
                dst3.ap = mybir.VecI64Pair([[YROW, 128], [1, YROW]])
                dst3.offset = b * OB
                (nc.gpsimd if b % 2 == 0 else nc.scalar).dma_start(out=dst3, in_=Y[:, :])

            # -- tail stores on the scalar queue behind the odd bulks:
            #    their loads have landed, their packets drain inside
            #    the bulk stream.
            for b in range(B):
                dstT = out[:].copy()
                dstT.ap = mybir.VecI64Pair([[FL, 128], [1, FL]])
                dstT.offset = b * OB + (NBULK - 31) * FL
                nc.scalar.dma_start(out=dstT, in_=TBs[b][:, :])

    nc.finalize()
    return nc


def run_sharded(x: np.ndarray, trace: bool = False):
    """Shard batch across 8 cores, run, gather. Returns (out, raw results)."""
    if "nc" not in _cache:
        _cache["nc"] = build_nc()
    nc = _cache["nc"]

    x = np.ascontiguousarray(x, dtype=np.float32)
    in_maps = [{"x": x[i * B : (i + 1) * B]} for i in range(N_CORES)]
    res = run_bass_kernel_spmd(nc, in_maps, list(range(N_CORES)), trace=trace)
    out = np.concatenate([res.results[i]["out"] for i in range(N_CORES)], axis=0)
    return out, res


def kernel(x: np.ndarray) -> np.ndarray:
    out, _ = run_sharded(x, trace=False)
    return out


# revision 11
# speedup vs baseline: 1.0869x; 1.0869x over previous
"""Trainium2 Bass kernel for sliding-window unfold (im2col).

reference:  out = x[:, idx, :]  with idx[w, f] = w + f
  x:   [128, 4096, 4]  f32
  out: [128, 4065, 32, 4]  f32

out[b, w] (= 128 floats = 512 B) is the contiguous slice
x[b].flat[4w : 4w + 128]; HBM write bandwidth is the roofline.

Measured on TRN2 (trace analysis across runs):
  - dma_starts spanning EXACTLY 128 SBUF partitions spray across all
    16 SDMA engines; DRAM->DRAM DMAs land on ONE engine (never again).
  - queues dispatch ~16 ns/descriptor (a 128-row DMA ~2.05 us of queue
    time); each DMA_DIRECT2D costs ~1.7 us on the issuing engine.
  - single-queue stores develop a ~20% straggler engine; splitting
    stores across two queues equalizes all 16 engines (v4).
  - DVE expand cost is nonlinear in windows: 31-window copy 5.1 us,
    16-window 1.22 us.  Two half-expands on DVE = ~2.4 us/batch.
  - HWDGE store issues can stall opaquely when many DMAs are in
    flight, so the scalar queue carries ONLY the 8 odd bulk stores.

Per batch b on each core (pure data parallel, 16 batches/core):
  1. load X[128, 248]: partition p holds x[b].flat[124p : 124p+248]
     (sync queue, all 16 X loads first - they gate everything).
  2. expand X -> Y[128, 3968] (Y[p, 128j+i] = X[p, 4j+i]) as two
     sequential DVE half-copies (16 + 15 windows).
  3. bulk store Y -> out[b] windows 0..3967: even b via gpsimd SWDGE,
     odd b via scalar HWDGE - two parallel store streams.
  4. tail windows 3937..4064: tail tiles [128, 128] load via gpsimd
     SWDGE descriptors interleaved between bulk gens (dependency-free,
     x is input); tail stores ride the sync queue after the X loads
     and drain inside the bulk stream.  31 duplicate rows rewrite bulk
     output with identical bytes, keeping the 128-partition fast path.
"""

import numpy as np

from concourse import bacc, mybir, tile
from concourse.bass_utils import run_bass_kernel_spmd

N_CORES = 8
B_FULL = 128
B = B_FULL // N_CORES  # 16 batches per core
S = 4096
C = 4
F = 32
W = S - F + 1    # 4065
FL = F * C       # 128 floats per window
XB = S * C       # 16384 floats per batch of x
OB = W * FL      # 520320 floats per batch of out
WPP = 31         # windows per partition in the bulk store
NBULK = 128 * WPP          # 3968 bulk windows per batch
NTAIL = W - NBULK          # 97 tail windows
YROW = WPP * FL            # 3968 floats per partition row
XROW = (WPP - 1) * C + FL  # 248 floats of x per partition
WSPLIT = 16                # windows per DVE half-expand

_cache = {}


def build_nc():
    nc = bacc.Bacc("TRN2", target_bir_lowering=False)
    x = nc.dram_tensor("x", [B, S, C], mybir.dt.float32, kind="ExternalInput")
    out = nc.dram_tensor("out", [B, W, F, C], mybir.dt.float32, kind="ExternalOutput")

    with tile.TileContext(nc) as tc:
        with (
            tc.tile_pool(name="xp", bufs=8) as xp,
            tc.tile_pool(name="yp", bufs=12) as yp,
            tc.tile_pool(name="tp", bufs=16) as tp,
        ):
            # -- X loads on the sync queue, compressed first.
            Xs = []
            for b in range(B):
                X = xp.tile([128, XROW], mybir.dt.float32)
                src = x[:].copy()
                src.ap = mybir.VecI64Pair([[WPP * C, 128], [1, XROW]])
                src.offset = b * XB
                nc.sync.dma_start(out=X[:, :], in_=src)
                Xs.append(X)

            TBs = [None] * B

            def tail_load(b):
                TB = tp.tile([128, FL], mybir.dt.float32)
                srcT = x[:].copy()
                srcT.ap = mybir.VecI64Pair([[C, 128], [1, FL]])
                srcT.offset = b * XB + (NBULK - 31) * C
                nc.gpsimd.dma_start(out=TB[:, :], in_=srcT)
                TBs[b] = TB

            # -- expand (2x DVE halves) + two-stream bulk stores; tail
            #    loads ride gpsimd between bulk gens (no dependencies).
            n_tl = 0
            for b in range(B):
                X = Xs[b]
                Y = yp.tile([128, YROW], mybir.dt.float32)

                srcA = X[:].copy()
                srcA.ap = mybir.VecI64Pair([[XROW, 128], [C, WSPLIT], [1, FL]])
                srcA.offset = 0
                dstA = Y[:].copy()
                dstA.ap = mybir.VecI64Pair([[YROW, 128], [FL, WSPLIT], [1, FL]])
                dstA.offset = 0
                nc.vector.tensor_copy(out=dstA, in_=srcA)

                srcB = X[:].copy()
                srcB.ap = mybir.VecI64Pair([[XROW, 128], [C, WPP - WSPLIT], [1, FL]])
                srcB.offset = WSPLIT * C
                dstB = Y[:].copy()
                dstB.ap = mybir.VecI64Pair([[YROW, 128], [FL, WPP - WSPLIT], [1, FL]])
                dstB.offset = WSPLIT * FL
                nc.vector.tensor_copy(out=dstB, in_=srcB)

                dst3 = out[:].copy()
                dst3.ap = mybir.VecI64Pair([[YROW, 128], [1, YROW]])
                dst3.offset = b * OB
                (nc.gpsimd if b % 2 == 0 else nc.scalar).dma_start(out=dst3, in_=Y[:, :])

                if b % 2 == 0:
                    tail_load(n_tl)
                    n_tl += 1
                    tail_load(n_tl)
                    n_tl += 1

            # -- tail stores on the sync queue after the X loads; their
            #    tiles landed via gpsimd long before each issue.
            for b in range(B):
                dstT = out[:].copy()
                dstT.ap = mybir.VecI64Pair([[FL, 128], [1, FL]])
                dstT.offset = b * OB + (NBULK - 31) * FL
                nc.sync.dma_start(out=dstT, in_=TBs[b][:, :])

    nc.finalize()
    return nc


def run_sharded(x: np.ndarray, trace: bool = False):
    """Shard batch across 8 cores, run, gather. Returns (out, raw results)."""
    if "nc" not in _cache:
        _cache["nc"] = build_nc()
    nc = _cache["nc"]

    x = np.ascontiguousarray(x, dtype=np.float32)
    in_maps = [{"x": x[i * B : (i + 1) * B]} for i in range(N_CORES)]
    res = run_bass_kernel_spmd(nc, in_maps, list(range(N_CORES)), trace=trace)
    out = np.concatenate([res.results[i]["out"] for i in range(N_CORES)], axis=0)
    return out, res


def kernel(x: np.ndarray) -> np.ndarray:
    out, _ = run_sharded(x, trace=False)
    return out
